# revision 5
# baseline (speedup 1.0000x reference)
"""Trainium2 Bass kernel for the nn_Dynamics problem.

Math (per batch element, d=8, H=128):
  x = X[:, :8], v = X[:, 8:]
  z0 = W0 x + b0; h0 = tanh(z0); z1 = W1 h0 + b1; h1 = tanh(z1)
  a1 = (1-h1^2)*w2;  A0 = W1^T a1;  a0 = (1-h0^2)*A0;  g = W0^T a0
  t0 = W0 v; h0p = (1-h0^2) t0; t1 = W1 h0p; u = h0 (1-h0^2) t0^2
  hvv = sum_h [-2*a1*h1*t1^2 - 2*A0*u]
  force = -(K x + D v)
  out = force - g * (g.force + hvv) / (1 + |g|^2)   (Sherman-Morrison)

Sign convention (saves ops; primed = negated): h0sq = h0^2
  h0p' = (h0sq-1) t0 = -h0p; t1' = -t1; a0' = -a0; g' = -g
  q = h0*a0' ; e2' = q*t0^2 = -e2; e12 = e1 - e2'
  hvv = -2 sum(e12);  num = hvv - g'.p';  out = p' + num/(1+gg) * g'

Layout: features on partitions, batch on the free axis, tiles of 512.
X is transposed to [16, B] on the HOST (f16), so no PE input transposes.
The per-element scalar "tail" runs batch-major after XBAR DMA transposes
of the packed [force; g'; hvv] block, batched over groups of 4 tiles.

Engine split per tile: PE 8 matmul streams; Act 2 tanh + 2 casts;
DVE customs (a1, e1) + 2x/4x STT/TT ops + tail; Pool t0f cast, t0^2, e12.

Sharding: pure data parallel over 8 NeuronCores (8192 rows each), weights
replicated, outputs concatenated.
"""

import os

import ml_dtypes
import numpy as np

import concourse.bacc as bacc
import concourse.bass as bass
import concourse.dve_ops as dve_ops
import concourse.tile as tile
from concourse import mybir
from concourse.bass_utils import run_bass_kernel_spmd
from concourse.dve_ops import DveOp
from concourse.dve_ops import has_src1
from concourse.dve_spec import C0, C1, C2, One, Spec, Src0, Src1, lower, sq
from concourse.dve_uop import DveOpSpec

F32 = mybir.dt.float32
F16 = mybir.dt.float16
AX = mybir.AxisListType
OP = mybir.AluOpType
ACT = mybir.ActivationFunctionType

DIM = 8
H = 128
BATCH = 65536
NCORES = 8
BC = BATCH // NCORES          # 8192 rows per core
TW = 512                      # batch tile width
NT = BC // TW                 # 16 tiles per core
NCH = TW // 128               # 4 chunks of 128 per tile
G = 4                         # tiles per tail group
NG = NT // G                  # 4 groups
CPG = G * NCH                 # 16 chunks per group
FMW = 96                      # fm/bm packed rows (force@0, g@32, hvv@64)

LAST_RESULTS = None

# ---------------- custom fused DVE ops ----------------


def _register_op(name, body, reference):
    if name in dve_ops._SUB_OPCODE_FOR_NAME:
        for op in dve_ops.OPS:
            if op.name == name:
                return op
    spec = Spec(body=body, reference=reference)
    shas = {}
    for ver in ("v3", "v4"):
        shas[ver] = DveOpSpec(
            name=name,
            opcode=dve_ops._CUSTOM_DVE_ROW_BASE + len(dve_ops.OPS),
            uops=lower(spec, ver=ver),
            rd1_en=has_src1(spec),
        ).sha(ver)
    op = DveOp(name, spec, subdim=False, uops_sha=shas)
    dve_ops.OPS.append(op)
    dve_ops.CUSTOM_DVE_SPECS[name] = spec
    dve_ops._SUB_OPCODE_FOR_NAME[name] = (
        dve_ops._CUSTOM_DVE_ROW_BASE + len(dve_ops.OPS) - 1
    )
    return op


# h0p' = (h0^2 - 1) * t0     (also a0' = (h0^2 - 1) * A0)
OP_SQM1_MUL = _register_op(
    "ANT_SQM1_MUL",
    (sq(Src0) - One) * Src1,
    lambda in0, in1: (in0 * in0 - 1.0) * in1,
)
# u' = h0 * (h0^2 - 1) * t0^2
OP_UPRIME = _register_op(
    "ANT_UPRIME",
    Src0 * (sq(Src0) - One) * sq(Src1),
    lambda in0, in1: in0 * (in0 * in0 - 1.0) * in1 * in1,
)
# e1 = (1 - h1^2) * w2 * h1 * t1^2
OP_E1F = _register_op(
    "ANT_E1F",
    (One - sq(Src0)) * C0 * Src0 * sq(Src1),
    lambda in0, in1, s0: (1.0 - in0 * in0) * s0 * in0 * in1 * in1,
)
# a1 = (1 - h1^2) * w2
OP_A1F = _register_op(
    "ANT_A1F",
    (One - sq(Src0)) * C0,
    lambda in0, s0: (1.0 - in0 * in0) * s0,
)


def build_nc():
    nc = bacc.Bacc()

    XT = nc.dram_tensor("XT", [2 * DIM, BC], F16, kind="ExternalInput")
    W0Tx = nc.dram_tensor("W0Tx", [2 * DIM, H], F16, kind="ExternalInput")
    W0Tv = nc.dram_tensor("W0Tv", [2 * DIM, H], F16, kind="ExternalInput")
    W0r = nc.dram_tensor("W0r", [H, 32], F16, kind="ExternalInput")
    W1 = nc.dram_tensor("W1", [H, H], F16, kind="ExternalInput")
    W1T = nc.dram_tensor("W1T", [H, H], F16, kind="ExternalInput")
    KDTn = nc.dram_tensor("KDTn", [2 * DIM, DIM], F16, kind="ExternalInput")
    m2red = nc.dram_tensor("m2red", [H, 32], F16, kind="ExternalInput")
    b0c = nc.dram_tensor("b0c", [H, 1], F32, kind="ExternalInput")
    b1c = nc.dram_tensor("b1c", [H, 1], F32, kind="ExternalInput")
    w2c = nc.dram_tensor("w2c", [H, 1], F32, kind="ExternalInput")
    out = nc.dram_tensor("out", [BC, DIM], F32, kind="ExternalOutput")
    # out natural order: batch b = 512*t + 128*c + p  ->  row (j p), j = 4t+c
    out_r = out.rearrange("(j p) f -> p j f", p=128)

    from contextlib import ExitStack

    with tile.TileContext(nc) as tc, ExitStack() as stk:
        consts = stk.enter_context(tc.tile_pool(name="consts", bufs=1))
        work = stk.enter_context(tc.tile_pool(name="work", bufs=2))
        xtp = stk.enter_context(tc.tile_pool(name="xtp", bufs=3))
        bmp = stk.enter_context(tc.tile_pool(name="bmp", bufs=2))
        obp = stk.enter_context(tc.tile_pool(name="obp", bufs=2))
        pzz = stk.enter_context(tc.tile_pool(name="pzz", bufs=2, space="PSUM"))
        ptt = stk.enter_context(tc.tile_pool(name="ptt", bufs=2, space="PSUM"))
        pA0 = stk.enter_context(tc.tile_pool(name="pA0", bufs=2, space="PSUM"))
        pfm = stk.enter_context(tc.tile_pool(name="pfm", bufs=2, space="PSUM"))

        # ---------------- constants ----------------
        W0Tx_sb = consts.tile([2 * DIM, H], F16)
        nc.sync.dma_start(out=W0Tx_sb, in_=W0Tx[:, :])
        W0Tv_sb = consts.tile([2 * DIM, H], F16)
        nc.sync.dma_start(out=W0Tv_sb, in_=W0Tv[:, :])
        W0_sb = consts.tile([H, 32], F16)
        nc.sync.dma_start(out=W0_sb, in_=W0r[:, :])
        W1_sb = consts.tile([H, H], F16)
        nc.sync.dma_start(out=W1_sb, in_=W1[:, :])
        W1T_sb = consts.tile([H, H], F16)
        nc.sync.dma_start(out=W1T_sb, in_=W1T[:, :])
        KDTn_sb = consts.tile([2 * DIM, DIM], F16)
        nc.sync.dma_start(out=KDTn_sb, in_=KDTn[:, :])
        m2_sb = consts.tile([H, 32], F16)
        nc.sync.dma_start(out=m2_sb, in_=m2red[:, :])
        b0_sb = consts.tile([H, 1], F32)
        nc.sync.dma_start(out=b0_sb, in_=b0c[:, :])
        b1_sb = consts.tile([H, 1], F32)
        nc.sync.dma_start(out=b1_sb, in_=b1c[:, :])
        w2_sb = consts.tile([H, 1], F32)
        nc.sync.dma_start(out=w2_sb, in_=w2c[:, :])

        # ---------------- main loop ----------------
        for g in range(NG):
            bm = bmp.tile([128, CPG * FMW], F16, tag="bm")
            for ti in range(G):
                t = G * g + ti
                XTs = xtp.tile([2 * DIM, TW], F16, tag="xt")
                nc.sync.dma_start(out=XTs, in_=XT[:, TW * t : TW * (t + 1)])

                z0 = pzz.tile([H, TW], F32, tag="zz")
                nc.tensor.matmul(z0, W0Tx_sb, XTs, start=True, stop=True)
                t0 = ptt.tile([H, TW], F32, tag="tt")
                nc.tensor.matmul(t0, W0Tv_sb, XTs, start=True, stop=True)

                h0 = work.tile([H, TW], F16)
                nc.scalar.activation(h0, z0, ACT.Tanh, bias=b0_sb, scale=1.0)

                # t0 -> f16 on Act (Pool cannot touch PSUM), t0^2 on Pool
                t0f = work.tile([H, TW], F16)
                nc.scalar.copy(t0f, t0)
                tsq0 = work.tile([H, TW], F16)
                nc.gpsimd.tensor_mul(tsq0, t0f, t0f)

                h0sq = work.tile([H, TW], F16)
                nc.vector.tensor_mul(h0sq, h0, h0)
                h0p = work.tile([H, TW], F16)
                nc.vector.scalar_tensor_tensor(
                    h0p, h0sq, 1.0, t0f, OP.subtract, OP.mult
                )

                z1 = pzz.tile([H, TW], F32, tag="zz")
                nc.tensor.matmul(z1, W1T_sb, h0, start=True, stop=True)
                t1 = ptt.tile([H, TW], F32, tag="tt")
                nc.tensor.matmul(t1, W1T_sb, h0p, start=True, stop=True)

                h1 = work.tile([H, TW], F16)
                nc.scalar.activation(h1, z1, ACT.Tanh, bias=b1_sb, scale=1.0)

                # a1 = (1-h1^2)*w2 ; e1 = a1*h1*t1^2
                a1 = work.tile([H, TW], F16)
                nc.vector._custom_dve(OP_A1F, out=a1, in0=h1, s0=w2_sb[:, 0:1])
                e1 = work.tile([H, TW], F16)
                nc.vector._custom_dve(
                    OP_E1F, out=e1, in0=h1, in1=t1[:, :], s0=w2_sb[:, 0:1]
                )

                A0 = pA0.tile([H, TW], F32, tag="A0")
                nc.tensor.matmul(A0, W1_sb, a1, start=True, stop=True)
                A0f = work.tile([H, TW], F16)
                nc.scalar.copy(A0f, A0)

                # a0' = (h0^2-1)*A0 ; q = h0*a0' ; e2' = q*t0^2
                a0 = work.tile([H, TW], F16)
                nc.vector.scalar_tensor_tensor(
                    a0, h0sq, 1.0, A0f, OP.subtract, OP.mult
                )
                q = work.tile([H, TW], F16)
                nc.vector.tensor_mul(q, h0, a0)
                e2 = work.tile([H, TW], F16)
                nc.vector.scalar_tensor_tensor(e2, q, 1.0, tsq0, OP.mult, OP.mult)

                # e12 = e1 - e2' on Pool; hvv = -2 sum(e12) via PE
                e12 = work.tile([H, TW], F16)
                nc.gpsimd.tensor_sub(e12, e1, e2)

                # feature-major packed block: p' rows 0:8, g' rows 32:40,
                # hvv row 64
                fm = pfm.tile([FMW, TW], F32, tag="fm")
                nc.tensor.matmul(
                    fm[0:DIM, :], KDTn_sb, XTs, start=True, stop=True
                )
                nc.tensor.matmul(
                    fm[32:64, :], W0_sb, a0, start=True, stop=True,
                    tile_position=(0, 32),
                )
                nc.tensor.matmul(
                    fm[64:96, :], m2_sb, e12, start=True, stop=True,
                    tile_position=(0, 64),
                )

                E = work.tile([FMW, TW], F16)
                nc.scalar.copy(E, fm[:, :])

                # XBAR DMA transpose to batch-major, packed into the group tile
                for c in range(NCH):
                    j = NCH * ti + c
                    nc.sync.dma_start_transpose(
                        out=bm[:, FMW * j : FMW * (j + 1)],
                        in_=E[:, 128 * c : 128 * (c + 1)],
                    )

            # ---------------- batched tail over CPG chunks ----------------
            def col3(off, w):
                return bass.AP(
                    tensor=bm.tensor,
                    offset=bm.offset + off,
                    ap=[list(bm.ap[0]), [FMW, CPG], [1, w]],
                )

            p3 = col3(0, DIM)
            g3 = col3(32, DIM)
            hv2 = bass.AP(
                tensor=bm.tensor,
                offset=bm.offset + 64,
                ap=[list(bm.ap[0]), [FMW, CPG]],
            )

            gb = work.tile([128, 2 * CPG * DIM], F16, tag="gb")
            gb3 = gb.rearrange("p (q c f) -> p (q c) f", f=DIM, q=2)
            nc.vector.scalar_tensor_tensor(
                gb3[:, 0:CPG, :], g3, 1.0, g3, OP.mult, OP.mult
            )
            nc.vector.scalar_tensor_tensor(
                gb3[:, CPG : 2 * CPG, :], g3, 1.0, p3, OP.mult, OP.mult
            )
            red = work.tile([128, 2 * CPG], F32, tag="red")
            nc.vector.tensor_reduce(red, gb3, axis=AX.X, op=OP.add)
            den = work.tile([128, CPG], F32, tag="den")
            nc.vector.tensor_scalar_add(den, red[:, 0:CPG], 1.0)
            gps = red[:, CPG : 2 * CPG]
            num = work.tile([128, CPG], F32, tag="num")
            nc.vector.tensor_sub(num, hv2, gps)
            rec = work.tile([128, CPG], F32, tag="rec")
            nc.vector.reciprocal(rec, den)
            s4 = work.tile([128, CPG], F32, tag="s4")
            nc.vector.tensor_mul(s4, num, rec)
            s4b = bass.AP(
                tensor=s4.tensor,
                offset=s4.offset,
                ap=[list(s4.ap[0]), [1, CPG], [0, DIM]],
            )
            su = work.tile([128, CPG * DIM], F32, tag="su")
            su3 = su.rearrange("p (c f) -> p c f", f=DIM)
            nc.vector.tensor_mul(su3, g3, s4b)
            ob = obp.tile([128, CPG * DIM], F32, tag="ob")
            nc.vector.tensor_add(
                ob.rearrange("p (c f) -> p c f", f=DIM), p3, su3
            )
            nc.sync.dma_start(
                out=out_r[:, CPG * g : CPG * (g + 1), :], in_=ob
            )

    if not nc.is_finalized():
        nc.finalize()

    return nc


_NC_CACHE = None


def _install_ntff_shim():
    """Register the axon NTFF profile hook (missing antenv.axon_hooks shim)."""
    import sys
    import types

    if "antenv.axon_hooks" in sys.modules:
        return
    try:
        sys.path.insert(0, "/root/.axon_site")
        from trn_agent_boot.trn_boot import _ntff_profile_via_ctypes

        hook = _ntff_profile_via_ctypes("/opt/axon/libaxon_pjrt.so")
        mod = types.ModuleType("antenv.axon_hooks")
        mod.get_axon_ntff_profile_hook = lambda: hook
        sys.modules["antenv.axon_hooks"] = mod
    except Exception:
        pass


def kernel(**inputs):
    global LAST_RESULTS, _NC_CACHE
    trace = bool(int(os.environ.get("KERNEL_TRACE", "0")))
    if trace:
        _install_ntff_shim()
    if _NC_CACHE is None:
        _NC_CACHE = build_nc()
    nc = _NC_CACHE

    X = np.ascontiguousarray(inputs["X"], dtype=np.float32)
    K = np.asarray(inputs["K"], np.float32)
    D = np.asarray(inputs["D"], np.float32)
    W0 = np.asarray(inputs["W0"], np.float32)
    W1 = np.asarray(inputs["W1"], np.float32)
    W2 = np.asarray(inputs["W2"], np.float32)
    w0pad = np.zeros((H, 32), np.float32)
    w0pad[:, 0:DIM] = W0
    w0tx = np.zeros((2 * DIM, H), np.float32)
    w0tx[0:DIM] = W0.T
    w0tv = np.zeros((2 * DIM, H), np.float32)
    w0tv[DIM:] = W0.T
    m2 = np.zeros((H, 32), np.float32)
    m2[:, 0] = -2.0
    shared = {
        "W0Tx": w0tx.astype(np.float16),
        "W0Tv": w0tv.astype(np.float16),
        "W0r": w0pad.astype(np.float16),
        "W1": W1.astype(np.float16),
        "W1T": np.ascontiguousarray(W1.T).astype(np.float16),
        "KDTn": np.ascontiguousarray(
            np.concatenate([-K.T, -D.T], axis=0)
        ).astype(np.float16),
        "m2red": m2.astype(np.float16),
        "b0c": np.asarray(inputs["b0"], np.float32).reshape(H, 1).copy(),
        "b1c": np.asarray(inputs["b1"], np.float32).reshape(H, 1).copy(),
        "w2c": W2.reshape(H, 1).copy(),
    }
    in_maps = []
    for i in range(NCORES):
        xt = np.ascontiguousarray(X[i * BC : (i + 1) * BC].T).astype(np.float16)
        m = {"XT": xt}
        m.update(shared)
        in_maps.append(m)

    res = run_bass_kernel_spmd(
        nc, in_maps, core_ids=list(range(NCORES)), trace=trace
    )
    LAST_RESULTS = res
    out_full = np.concatenate(
        [res.results[i]["out"] for i in range(NCORES)], axis=0
    )
    return out_full.astype(np.float32)


# revision 14
# speedup vs baseline: 1.5291x; 1.5291x over previous
"""Trainium2 Bass kernel for the nn_Dynamics problem.

Math (per batch element, d=8, H=128):
  x = X[:, :8], v = X[:, 8:]
  z0 = W0 x + b0; h0 = tanh(z0); z1 = W1 h0 + b1; h1 = tanh(z1)
  a1 = (1-h1^2)*w2;  A0 = W1^T a1;  a0 = (1-h0^2)*A0;  g = W0^T a0
  t0 = W0 v; h0p = (1-h0^2) t0; t1 = W1 h0p; u = h0 (1-h0^2) t0^2
  hvv = sum_h [-2*a1*h1*t1^2 - 2*A0*u]
  force = -(K x + D v)
  out = force - g * (g.force + hvv) / (1 + |g|^2)   (Sherman-Morrison)

Sign convention (saves ops; primed = negated): h0sq = h0^2
  h0p' = (h0sq-1) t0 = -h0p; t1' = -t1; a0' = -a0; g' = -g
  q = h0*a0' ; e2' = q*t0^2 = -e2; e12 = e1 - e2'
  hvv = -2 sum(e12);  num = hvv - g'.p';  out = p' + num/(1+gg) * g'

Layout: features on partitions, batch on the free axis, tiles of 512.
X is transposed to [16, B] on the HOST (f16), so no PE input transposes.
The per-element scalar "tail" runs batch-major after XBAR DMA transposes
of the packed [force; g'; hvv] block, batched over groups of 4 tiles.

Engine split per tile: PE 8 matmul streams; Act 2 tanh + 2 casts;
DVE customs (a1, e1) + 2x/4x STT/TT ops + tail; Pool t0f cast, t0^2, e12.

Sharding: pure data parallel over 8 NeuronCores (8192 rows each), weights
replicated, outputs concatenated.
"""

import os

import ml_dtypes
import numpy as np

import concourse.bacc as bacc
import concourse.bass as bass
import concourse.dve_ops as dve_ops
import concourse.tile as tile
from concourse import mybir
from concourse.bass_utils import run_bass_kernel_spmd
from concourse.dve_ops import DveOp
from concourse.dve_ops import has_src1
from concourse.dve_spec import C0, C1, C2, One, Spec, Src0, Src1, lower, sq
from concourse.dve_uop import DveOpSpec
from concourse.masks import make_identity

F32 = mybir.dt.float32
F16 = mybir.dt.float16
AX = mybir.AxisListType
OP = mybir.AluOpType
ACT = mybir.ActivationFunctionType

DIM = 8
H = 128
BATCH = 65536
NCORES = 8
BC = BATCH // NCORES          # 8192 rows per core
TW = 512                      # batch tile width
NT = BC // TW                 # 16 tiles per core
NCH = TW // 128               # 4 chunks of 128 per tile
G = 4                         # tiles per tail group
NG = NT // G                  # 4 groups
CPG = G * NCH                 # 16 chunks per group
FMW = 96                      # fm/bm packed rows (force@0, g@32, hvv@64)

LAST_RESULTS = None

# ---------------- custom fused DVE ops ----------------


def _register_op(name, body, reference):
    if name in dve_ops._SUB_OPCODE_FOR_NAME:
        for op in dve_ops.OPS:
            if op.name == name:
                return op
    spec = Spec(body=body, reference=reference)
    shas = {}
    for ver in ("v3", "v4"):
        shas[ver] = DveOpSpec(
            name=name,
            opcode=dve_ops._CUSTOM_DVE_ROW_BASE + len(dve_ops.OPS),
            uops=lower(spec, ver=ver),
            rd1_en=has_src1(spec),
        ).sha(ver)
    op = DveOp(name, spec, subdim=False, uops_sha=shas)
    dve_ops.OPS.append(op)
    dve_ops.CUSTOM_DVE_SPECS[name] = spec
    dve_ops._SUB_OPCODE_FOR_NAME[name] = (
        dve_ops._CUSTOM_DVE_ROW_BASE + len(dve_ops.OPS) - 1
    )
    return op


# h0p' = (h0^2 - 1) * t0     (also a0' = (h0^2 - 1) * A0)
OP_SQM1_MUL = _register_op(
    "ANT_SQM1_MUL",
    (sq(Src0) - One) * Src1,
    lambda in0, in1: (in0 * in0 - 1.0) * in1,
)
# u' = h0 * (h0^2 - 1) * t0^2
OP_UPRIME = _register_op(
    "ANT_UPRIME",
    Src0 * (sq(Src0) - One) * sq(Src1),
    lambda in0, in1: in0 * (in0 * in0 - 1.0) * in1 * in1,
)
# e1 = (1 - h1^2) * w2 * h1 * t1^2
OP_E1F = _register_op(
    "ANT_E1F",
    (One - sq(Src0)) * C0 * Src0 * sq(Src1),
    lambda in0, in1, s0: (1.0 - in0 * in0) * s0 * in0 * in1 * in1,
)
# a1 = (1 - h1^2) * w2
OP_A1F = _register_op(
    "ANT_A1F",
    (One - sq(Src0)) * C0,
    lambda in0, s0: (1.0 - in0 * in0) * s0,
)


def build_nc():
    nc = bacc.Bacc()

    XT = nc.dram_tensor("XT", [2 * DIM, BC], F16, kind="ExternalInput")
    W0Tx = nc.dram_tensor("W0Tx", [2 * DIM, H], F16, kind="ExternalInput")
    W0Tv = nc.dram_tensor("W0Tv", [2 * DIM, H], F16, kind="ExternalInput")
    W0r = nc.dram_tensor("W0r", [H, 32], F16, kind="ExternalInput")
    W1w = nc.dram_tensor("W1w", [H, H], F16, kind="ExternalInput")
    W1T = nc.dram_tensor("W1T", [H, H], F16, kind="ExternalInput")
    KDTn = nc.dram_tensor("KDTn", [2 * DIM, DIM], F16, kind="ExternalInput")
    m2red = nc.dram_tensor("m2red", [H, 32], F16, kind="ExternalInput")
    b0c = nc.dram_tensor("b0c", [H, 1], F32, kind="ExternalInput")
    b1c = nc.dram_tensor("b1c", [H, 1], F32, kind="ExternalInput")
    cvec = nc.dram_tensor("cvec", [H, 1], F32, kind="ExternalInput")
    w2c = nc.dram_tensor("w2c", [H, 1], F32, kind="ExternalInput")
    out = nc.dram_tensor("out", [BC, DIM], F32, kind="ExternalOutput")
    # out natural order: batch b = 512*t + 128*c + p  ->  row (j p), j = 4t+c
    out_r = out.rearrange("(j p) f -> p j f", p=128)

    from contextlib import ExitStack

    with tile.TileContext(nc) as tc, ExitStack() as stk:
        consts = stk.enter_context(tc.tile_pool(name="consts", bufs=1))
        work = stk.enter_context(tc.tile_pool(name="work", bufs=2))
        xtp = stk.enter_context(tc.tile_pool(name="xtp", bufs=3))
        bmp = stk.enter_context(tc.tile_pool(name="bmp", bufs=1, space="PSUM"))
        obp = stk.enter_context(tc.tile_pool(name="obp", bufs=2))
        pzz = stk.enter_context(tc.tile_pool(name="pzz", bufs=2, space="PSUM"))
        ptt = stk.enter_context(tc.tile_pool(name="ptt", bufs=2, space="PSUM"))
        pA0 = stk.enter_context(tc.tile_pool(name="pA0", bufs=1, space="PSUM"))
        pfm = stk.enter_context(tc.tile_pool(name="pfm", bufs=1, space="PSUM"))

        # ---------------- constants ----------------
        W0Tx_sb = consts.tile([2 * DIM, H], F16)
        nc.sync.dma_start(out=W0Tx_sb, in_=W0Tx[:, :])
        W0Tv_sb = consts.tile([2 * DIM, H], F16)
        nc.sync.dma_start(out=W0Tv_sb, in_=W0Tv[:, :])
        W0_sb = consts.tile([H, 32], F16)
        nc.sync.dma_start(out=W0_sb, in_=W0r[:, :])
        W1w_sb = consts.tile([H, H], F16)
        nc.sync.dma_start(out=W1w_sb, in_=W1w[:, :])
        W1T_sb = consts.tile([H, H], F16)
        nc.sync.dma_start(out=W1T_sb, in_=W1T[:, :])
        KDTn_sb = consts.tile([2 * DIM, DIM], F16)
        nc.sync.dma_start(out=KDTn_sb, in_=KDTn[:, :])
        m2_sb = consts.tile([H, 32], F16)
        nc.sync.dma_start(out=m2_sb, in_=m2red[:, :])
        b0_sb = consts.tile([H, 1], F32)
        nc.sync.dma_start(out=b0_sb, in_=b0c[:, :])
        b1_sb = consts.tile([H, 1], F32)
        nc.sync.dma_start(out=b1_sb, in_=b1c[:, :])
        cvec_sb = consts.tile([H, 1], F32)
        nc.sync.dma_start(out=cvec_sb, in_=cvec[:, :])
        w2_sb = consts.tile([H, 1], F32)
        nc.sync.dma_start(out=w2_sb, in_=w2c[:, :])

        ident = consts.tile([128, 128], F32)
        make_identity(nc, ident)
        ident_h = consts.tile([128, 128], F16)
        nc.scalar.copy(ident_h, ident)

        # ---------------- main loop ----------------
        for g in range(NG):
            bm = bmp.tile([128, CPG * FMW], F16, tag="bm")
            for ti in range(G):
                t = G * g + ti
                XTs = xtp.tile([2 * DIM, TW], F16, tag="xt")
                nc.sync.dma_start(out=XTs, in_=XT[:, TW * t : TW * (t + 1)])

                z0 = pzz.tile([H, TW], F32, tag="zz")
                nc.tensor.matmul(z0, W0Tx_sb, XTs, start=True, stop=True)
                t0 = ptt.tile([H, TW], F32, tag="tt")
                nc.tensor.matmul(t0, W0Tv_sb, XTs, start=True, stop=True)

                h0 = work.tile([H, TW], F16)
                nc.scalar.activation(h0, z0, ACT.Tanh, bias=b0_sb, scale=1.0)

                # h0p' = (h0^2-1)*t0 ; u' = h0*(h0^2-1)*t0^2
                h0p = work.tile([H, TW], F16)
                nc.vector._custom_dve(OP_SQM1_MUL, out=h0p, in0=h0, in1=t0[:, :])
                u = work.tile([H, TW], F16)
                nc.vector._custom_dve(OP_UPRIME, out=u, in0=h0, in1=t0[:, :])

                z1 = pzz.tile([H, TW], F32, tag="zz")
                nc.tensor.matmul(z1, W1T_sb, h0, start=True, stop=True)
                t1 = ptt.tile([H, TW], F32, tag="tt")
                nc.tensor.matmul(t1, W1T_sb, h0p, start=True, stop=True)

                h1 = work.tile([H, TW], F16)
                nc.scalar.activation(h1, z1, ACT.Tanh, bias=b1_sb, scale=1.0)

                # e1 = (1-h1^2)*w2*h1*t1^2 ; A0 via folded W1w on h1^2
                h1sq = work.tile([H, TW], F16)
                nc.vector.tensor_mul(h1sq, h1, h1)
                e1 = work.tile([H, TW], F16)
                nc.vector._custom_dve(
                    OP_E1F, out=e1, in0=h1, in1=t1[:, :], s0=w2_sb[:, 0:1]
                )

                A0 = pA0.tile([H, TW], F32, tag="A0")
                nc.tensor.matmul(A0, W1w_sb, h1sq, start=True, stop=True)
                # A0_true = A0_partial + colsum(W1*w2) (Act bias add + cast)
                A0f = work.tile([H, TW], F16)
                nc.scalar.activation(
                    A0f, A0, ACT.Identity, bias=cvec_sb, scale=1.0
                )

                # a0' = (h0^2-1)*A0 ; e2' = A0*u'
                a0 = work.tile([H, TW], F16)
                nc.vector._custom_dve(OP_SQM1_MUL, out=a0, in0=h0, in1=A0f)
                e2 = work.tile([H, TW], F16)
                nc.gpsimd.tensor_mul(e2, A0f, u)

                # e12 = e1 - e2' on Pool; hvv = -2 sum(e12) via PE
                e12 = work.tile([H, TW], F16)
                nc.gpsimd.tensor_sub(e12, e1, e2)

                # feature-major packed block: p' rows 0:8, g' rows 32:40,
                # hvv row 64
                fm = pfm.tile([FMW, TW], F32, tag="fm")
                nc.tensor.matmul(
                    fm[0:DIM, :], KDTn_sb, XTs, start=True, stop=True
                )
                nc.tensor.matmul(
                    fm[32:64, :], W0_sb, a0, start=True, stop=True,
                    tile_position=(0, 32),
                )
                nc.tensor.matmul(
                    fm[64:96, :], m2_sb, e12, start=True, stop=True,
                    tile_position=(0, 64),
                )

                E = work.tile([FMW, TW], F16)
                nc.scalar.copy(E, fm[:, :])

                # PE f16 transpose to batch-major, packed into the group tile
                for c in range(NCH):
                    j = NCH * ti + c
                    nc.tensor.transpose(
                        bm[:, FMW * j : FMW * (j + 1)],
                        E[:, 128 * c : 128 * (c + 1)],
                        ident_h[0:FMW, 0:FMW],
                    )

            # ---------------- batched tail over CPG chunks ----------------
            def col3(off, w):
                return bass.AP(
                    tensor=bm.tensor,
                    offset=bm.offset + off,
                    ap=[list(bm.ap[0]), [FMW, CPG], [1, w]],
                )

            p3 = col3(0, DIM)
            g3 = col3(32, DIM)
            hv2 = bass.AP(
                tensor=bm.tensor,
                offset=bm.offset + 64,
                ap=[list(bm.ap[0]), [FMW, CPG]],
            )

            # g' columns to SBUF (DVE can't read 2 PSUM inputs per op)
            gS = work.tile([128, CPG * DIM], F16, tag="gS")
            gS3 = gS.rearrange("p (c f) -> p c f", f=DIM)
            nc.scalar.copy(gS3, g3)

            gb = work.tile([128, 2 * CPG * DIM], F16, tag="gb")
            gb3 = gb.rearrange("p (q c f) -> p (q c) f", f=DIM, q=2)
            nc.vector.tensor_mul(gb3[:, 0:CPG, :], gS3, gS3)
            nc.vector.tensor_mul(gb3[:, CPG : 2 * CPG, :], gS3, p3)
            red = work.tile([128, 2 * CPG], F32, tag="red")
            nc.vector.tensor_reduce(red, gb3, axis=AX.X, op=OP.add)
            den = work.tile([128, CPG], F32, tag="den")
            nc.vector.tensor_scalar_add(den, red[:, 0:CPG], 1.0)
            gps = red[:, CPG : 2 * CPG]
            num = work.tile([128, CPG], F32, tag="num")
            nc.vector.tensor_sub(num, hv2, gps)
            rec = work.tile([128, CPG], F32, tag="rec")
            nc.vector.reciprocal(rec, den)
            s4 = work.tile([128, CPG], F32, tag="s4")
            nc.vector.tensor_mul(s4, num, rec)
            s4b = bass.AP(
                tensor=s4.tensor,
                offset=s4.offset,
                ap=[list(s4.ap[0]), [1, CPG], [0, DIM]],
            )
            su = work.tile([128, CPG * DIM], F32, tag="su")
            su3 = su.rearrange("p (c f) -> p c f", f=DIM)
            nc.vector.tensor_mul(su3, gS3, s4b)
            ob = obp.tile([128, CPG * DIM], F32, tag="ob")
            nc.vector.tensor_add(
                ob.rearrange("p (c f) -> p c f", f=DIM), p3, su3
            )
            nc.sync.dma_start(
                out=out_r[:, CPG * g : CPG * (g + 1), :], in_=ob
            )

    if not nc.is_finalized():
        nc.finalize()

    return nc


_NC_CACHE = None


def _install_ntff_shim():
    """Register the axon NTFF profile hook (missing antenv.axon_hooks shim)."""
    import sys
    import types

    if "antenv.axon_hooks" in sys.modules:
        return
    try:
        sys.path.insert(0, "/root/.axon_site")
        from trn_agent_boot.trn_boot import _ntff_profile_via_ctypes

        hook = _ntff_profile_via_ctypes("/opt/axon/libaxon_pjrt.so")
        mod = types.ModuleType("antenv.axon_hooks")
        mod.get_axon_ntff_profile_hook = lambda: hook
        sys.modules["antenv.axon_hooks"] = mod
    except Exception:
        pass


def kernel(**inputs):
    global LAST_RESULTS, _NC_CACHE
    trace = bool(int(os.environ.get("KERNEL_TRACE", "0")))
    if trace:
        _install_ntff_shim()
    if _NC_CACHE is None:
        _NC_CACHE = build_nc()
    nc = _NC_CACHE

    X = np.ascontiguousarray(inputs["X"], dtype=np.float32)
    K = np.asarray(inputs["K"], np.float32)
    D = np.asarray(inputs["D"], np.float32)
    W0 = np.asarray(inputs["W0"], np.float32)
    W1 = np.asarray(inputs["W1"], np.float32)
    W2 = np.asarray(inputs["W2"], np.float32)
    w0pad = np.zeros((H, 32), np.float32)
    w0pad[:, 0:DIM] = W0
    w0tx = np.zeros((2 * DIM, H), np.float32)
    w0tx[0:DIM] = W0.T
    w0tv = np.zeros((2 * DIM, H), np.float32)
    w0tv[DIM:] = W0.T
    m2 = np.zeros((H, 32), np.float32)
    m2[:, 0] = -2.0
    w2v = W2.reshape(H)
    w1w = -(W1 * w2v[:, None])
    cv = (W1 * w2v[:, None]).sum(axis=0).astype(np.float32)
    shared = {
        "W0Tx": w0tx.astype(np.float16),
        "W0Tv": w0tv.astype(np.float16),
        "W0r": w0pad.astype(np.float16),
        "W1w": w1w.astype(np.float16),
        "W1T": np.ascontiguousarray(W1.T).astype(np.float16),
        "KDTn": np.ascontiguousarray(
            np.concatenate([-K.T, -D.T], axis=0)
        ).astype(np.float16),
        "m2red": m2.astype(np.float16),
        "b0c": np.asarray(inputs["b0"], np.float32).reshape(H, 1).copy(),
        "b1c": np.asarray(inputs["b1"], np.float32).reshape(H, 1).copy(),
        "cvec": np.ascontiguousarray(cv.reshape(H, 1)),
        "w2c": W2.reshape(H, 1).copy(),
    }
    in_maps = []
    for i in range(NCORES):
        xt = np.ascontiguousarray(X[i * BC : (i + 1) * BC].T).astype(np.float16)
        m = {"XT": xt}
        m.update(shared)
        in_maps.append(m)

    res = run_bass_kernel_spmd(
        nc, in_maps, core_ids=list(range(NCORES)), trace=trace
    )
    LAST_RESULTS = res
    out_full = np.concatenate(
        [res.results[i]["out"] for i in range(NCORES)], axis=0
    )
    return out_full.astype(np.float32)
